# revision 1
# baseline (speedup 1.0000x reference)
"""Bass/Tile TRN2 kernel for the attention module:

    pre    = prev_hidden @ W1[:H] + b1                    [B, H]
    hidden = tanh(pre[:, None, :] + ann @ W1[H:])         [B, S, H]
    score  = hidden @ W2 (+ b2; softmax-invariant, drop)  [B, S]
    alpha  = softmax(score, axis=1)
    ctx    = alpha @ ann                                  [B, 1, A]

B=32, S=4096, A=H=512. Sharding: data-parallel over batch, 4 batches per
core on 8 cores. Single pass over S per batch with an unnormalized
online softmax (scores are bounded: |score| <= sum|W2|+|b2| ~ 11.4, so
exp never overflows in fp32 and no running-max is needed):

    w_s = exp(score_s);  Z = sum w_s;  ctx = (sum w_s * ann_s) / Z

Layouts: the s-dim matmul (ann @ W1a) contracts over the feature dim a,
so it needs ann with a on SBUF partitions (annT); the context matmul
contracts over s, so it needs natural ann. Host supplies both layouts in
bf16 (same total HBM bytes as one fp32 copy).
"""

import os

import numpy as np
import ml_dtypes

B = 32
S = 4096
A = 512
H = 512
NCORES = 8
BL = B // NCORES  # 4 batches per core
SC = 512          # s-chunk processed per inner iteration
NSC = S // SC     # 8

BF16 = ml_dtypes.bfloat16

_BUILT = None       # (nc,) cache — Bass module is reusable across calls
LAST_RESULT = None  # last BassKernelResults, for test harness introspection

LDW_DEDUP = False  # dropping LDWs breaks LDW<->MM pairing (verified wrong results)

# Stage selection for HW attribution profiling (all on for the real kernel)
STAGES = {"dma", "step2", "tanh", "score", "exp", "transpose", "ctx"}


def _build_bass(loop_n=None):
    """Build the Bass module. loop_n wraps the main s-loop in a For_i
    executed loop_n times — a timing amplifier (outputs then meaningless);
    loop_n=None builds the real single-pass kernel."""
    from contextlib import ExitStack, nullcontext

    import concourse.bass as bass
    import concourse.tile as tile
    from concourse import bacc, mybir
    from concourse.masks import make_identity

    bf16 = mybir.dt.bfloat16
    f32 = mybir.dt.float32
    Tanh = mybir.ActivationFunctionType.Tanh
    Exp = mybir.ActivationFunctionType.Exp

    nc = bacc.Bacc()

    annT_d = nc.dram_tensor("annT", [BL, A, S], bf16, kind="ExternalInput")
    annN_d = nc.dram_tensor("annN", [BL, S, A], bf16, kind="ExternalInput")
    w1a_d = nc.dram_tensor("w1a", [A, H], bf16, kind="ExternalInput")
    w1h_d = nc.dram_tensor("w1h", [H, H], bf16, kind="ExternalInput")
    b1_d = nc.dram_tensor("b1", [1, H], bf16, kind="ExternalInput")
    w2_d = nc.dram_tensor("w2", [H, 32], bf16, kind="ExternalInput")
    pvt_d = nc.dram_tensor("pvt", [H, BL], bf16, kind="ExternalInput")
    out_d = nc.dram_tensor("out", [BL, A], f32, kind="ExternalOutput")

    with tile.TileContext(nc) as tc, ExitStack() as ctx:
        singles = ctx.enter_context(tc.tile_pool(name="singles", bufs=1))
        annt_pool = ctx.enter_context(tc.tile_pool(name="annt", bufs=4))
        annn_pool = ctx.enter_context(tc.tile_pool(name="annn", bufs=3))
        th_pool = ctx.enter_context(tc.tile_pool(name="thp", bufs=3))
        w_pool = ctx.enter_context(tc.tile_pool(name="wp", bufs=3))
        psum2 = ctx.enter_context(
            tc.tile_pool(name="psum2", bufs=1, space="PSUM")
        )
        psum_wc = ctx.enter_context(
            tc.tile_pool(name="psumwc", bufs=2, space="PSUM")
        )
        psum1 = ctx.enter_context(
            tc.tile_pool(name="psum1", bufs=1, space="PSUM")
        )

        # ---- constants / weights in SBUF ----
        ident = singles.tile([128, 128], bf16)
        make_identity(nc, ident)

        w1a_sb = singles.tile([128, 4, H], bf16)  # (a%128, a//128, h)
        nc.sync.dma_start(
            out=w1a_sb, in_=w1a_d[:, :].rearrange("(ac p) h -> p ac h", p=128)
        )
        w1h_sb = singles.tile([128, 4, H], bf16)  # (hin%128, hin//128, h)
        nc.sync.dma_start(
            out=w1h_sb, in_=w1h_d[:, :].rearrange("(kc p) h -> p kc h", p=128)
        )
        b1_sb = singles.tile([1, H], bf16)
        nc.sync.dma_start(out=b1_sb, in_=b1_d[:, :])
        # W2 replicated x32 so score matmuls write a full 32-row col group
        w2_sb = singles.tile([128, 4, 32], bf16)  # (h%128, h//128, rep)
        nc.sync.dma_start(
            out=w2_sb, in_=w2_d[:, :].rearrange("(hc p) r -> p hc r", p=128)
        )
        pvt_sb = singles.tile([128, 4, BL], bf16)  # (hin%128, hin//128, b)
        nc.sync.dma_start(
            out=pvt_sb, in_=pvt_d[:, :].rearrange("(kc p) b -> p kc b", p=128)
        )
        ones_sb = singles.tile([1, BL], bf16)
        nc.vector.memset(ones_sb, 1.0)

        # ---- pre2T[h, b] = (prev @ W1h).T + b1 broadcast, in PSUM ----
        pre_ps = psum2.tile([128, 4, BL], f32, tag="score")
        for hc in range(4):
            for kc in range(4):
                nc.tensor.matmul(
                    pre_ps[:, hc, :],
                    lhsT=w1h_sb[:, kc, hc * 128:(hc + 1) * 128],
                    rhs=pvt_sb[:, kc, :],
                    start=(kc == 0),
                    stop=False,
                )
            # b1 contribution: rank-1 with ones row (K=1)
            nc.tensor.matmul(
                pre_ps[:, hc, :],
                lhsT=b1_sb[:, hc * 128:(hc + 1) * 128],
                rhs=ones_sb[:, :],
                start=False,
                stop=True,
            )
        pre_sb = singles.tile([128, 4, BL], f32)
        nc.scalar.copy(out=pre_sb, in_=pre_ps)

        # ---- main streaming loop over s-chunks ----
        z_sb = singles.tile([128, NSC], f32)
        ctx_ps = psum1.tile([128, A], f32, tag="ctx")

        outer = (
            tc.For_i(0, loop_n, 1) if loop_n is not None else nullcontext()
        )
        with outer:
            _main_body(
                nc, tc, mybir,
                annT_d, annN_d, w1a_sb, w2_sb, pre_sb, ident,
                annt_pool, annn_pool, th_pool, w_pool, psum2, psum_wc,
                z_sb, ctx_ps,
            )

        # ---- normalize and store ----
        out_sb = singles.tile([128, A], f32)
        if "exp" in STAGES and "ctx" in STAGES:
            z_tot = singles.tile([128, 1], f32)
            nc.vector.reduce_sum(
                out=z_tot, in_=z_sb, axis=mybir.AxisListType.X
            )
            z_rec = singles.tile([128, 1], f32)
            nc.vector.reciprocal(out=z_rec, in_=z_tot)
            nc.vector.tensor_scalar_mul(out_sb, ctx_ps[:, :], z_rec)
        else:
            nc.vector.memset(out_sb, 0.0)
        nc.sync.dma_start(out=out_d[:, :], in_=out_sb[0:128:32, :])

    if LDW_DEDUP:
        _dedup_ldweights(nc, mybir)
    nc.finalize()
    return nc


def _dedup_ldweights(nc, mybir):
    """Drop InstLdweights whose weights AP is identical to the previous
    (kept) InstLdweights with no different load in between; waits/updates
    are spliced onto the following instruction."""
    for f in nc.m.functions:
        for blk in f.blocks:
            insts = list(blk.instructions)
            keep = []
            last_key = None
            pending_sync = None
            for inst in insts:
                tn = type(inst).__name__
                if tn == "InstLdweights":
                    key = str(inst.ins[0])
                    if key == last_key:
                        si = inst.sync_info
                        if si is not None and (si.on_wait or si.on_update):
                            if pending_sync is None:
                                pending_sync = ([], [])
                            pending_sync[0].extend(si.on_wait)
                            pending_sync[1].extend(si.on_update)
                        continue  # drop it
                    last_key = key
                elif tn == "InstMatmult":
                    pass  # matmuls don't invalidate loaded weights
                else:
                    pass  # other-engine instrs in the block don't touch PE
                if pending_sync is not None:
                    si = inst.sync_info
                    ow = list(pending_sync[0])
                    ou = list(pending_sync[1])
                    if si is not None:
                        ow += list(si.on_wait)
                        ou += list(si.on_update)
                    inst.sync_info = mybir.SyncInfo(on_wait=ow, on_update=ou)
                    pending_sync = None
                keep.append(inst)
            if len(keep) != len(insts):
                blk.instructions = keep


def _main_body(
    nc, tc, mybir,
    annT_d, annN_d, w1a_sb, w2_sb, pre_sb, ident,
    annt_pool, annn_pool, th_pool, w_pool, psum2, psum_wc,
    z_sb, ctx_ps,
):
    bf16 = mybir.dt.bfloat16
    f32 = mybir.dt.float32
    Tanh = mybir.ActivationFunctionType.Tanh
    Exp = mybir.ActivationFunctionType.Exp

    # Batch-inner matmul ordering: 4 consecutive MMs share the stationary
    # weight block (weight reloads are the dominant per-MM cost), and the
    # transpose+ctx tail of chunk sc-1 is deferred so its exp/DVE deps are
    # resolved before the PE reaches it.
    pend = None
    for sc in range(NSC + 1):
        if sc < NSC:
            score_ps = psum2.tile([128, SC], f32, tag="score")
            at_tiles, an_tiles, th_tiles, thp_tiles = [], [], [], []
            for b in range(BL):
                at_sb = annt_pool.tile([128, 4, SC], bf16, tag=f"at{b}")
                if "dma" in STAGES:
                    nc.sync.dma_start(
                        out=at_sb,
                        in_=annT_d[b, :, sc * SC:(sc + 1) * SC].rearrange(
                            "(ac p) s -> p ac s", p=128
                        ),
                    )
                else:
                    nc.vector.memset(at_sb[:, 0, 0:1], 0.5)
                at_tiles.append(at_sb)
                an_sb = annn_pool.tile([128, 4, A], bf16, tag=f"an{b}")
                if "dma" in STAGES:
                    nc.sync.dma_start(
                        out=an_sb,
                        in_=annN_d[b, sc * SC:(sc + 1) * SC, :].rearrange(
                            "(sb p) a -> p sb a", p=128
                        ),
                    )
                else:
                    nc.vector.memset(an_sb[:, 0, 0:1], 0.5)
                an_tiles.append(an_sb)
                th_sb = th_pool.tile([128, 4, SC], bf16, tag=f"th{b}")
                if "step2" not in STAGES or "tanh" not in STAGES:
                    nc.vector.memset(th_sb[:, 0, 0:1], 0.5)
                th_tiles.append(th_sb)
                thp = psum2.tile([128, SC], f32, tag=f"thp{b}")
                thp_tiles.append(thp)

            if "step2" in STAGES:
                for hc in range(4):
                    for ac in range(4):
                        for b in range(BL):
                            nc.tensor.matmul(
                                thp_tiles[b][:, :],
                                lhsT=w1a_sb[:, ac, hc * 128:(hc + 1) * 128],
                                rhs=at_tiles[b][:, ac, :],
                                start=(ac == 0),
                                stop=(ac == 3),
                            )
                    if "tanh" in STAGES:
                        for b in range(BL):
                            nc.scalar.activation(
                                out=th_tiles[b][:, hc, :],
                                in_=thp_tiles[b][:, :],
                                func=Tanh,
                                bias=pre_sb[:, hc, b:b + 1],
                                scale=1.0,
                            )
            else:
                for b in range(BL):
                    nc.vector.memset(thp_tiles[b][:, 0:1], 0.5)

            if "score" in STAGES:
                for hc in range(4):
                    for b in range(BL):
                        nc.tensor.matmul(
                            score_ps[32 * b:32 * b + 32, :],
                            lhsT=w2_sb[:, hc, :],
                            rhs=th_tiles[b][:, hc, :],
                            start=(hc == 0),
                            stop=(hc == 3),
                            tile_position=(0, 32 * b),
                        )
            else:
                nc.vector.memset(score_ps[:, 0:1], 0.5)

            w_sb = w_pool.tile([128, SC], bf16, tag="w")
            if "exp" in STAGES:
                nc.scalar.activation(
                    out=w_sb,
                    in_=score_ps[:, :],
                    func=Exp,
                    accum_out=z_sb[:, sc:sc + 1],
                )
            else:
                nc.vector.memset(w_sb[:, 0:1], 0.5)
        else:
            w_sb = None
            an_tiles = None

        if pend is not None:
            p_w, p_an, p_sc = pend
            wcol_sb = w_pool.tile([128, 4, 128], bf16, tag="wcol")
            if "transpose" in STAGES:
                for st in range(4):
                    wc_ps = psum_wc.tile([128, 128], bf16, tag="wc")
                    nc.tensor.transpose(
                        wc_ps[:, :], p_w[:, st * 128:(st + 1) * 128],
                        ident[:, :],
                    )
                    nc.vector.tensor_copy(
                        out=wcol_sb[:, st, :], in_=wc_ps[:, :]
                    )
            else:
                nc.vector.memset(wcol_sb[:, 0, 0:1], 0.5)
            if "ctx" in STAGES:
                # st outer / b inner: consecutive MMs target disjoint
                # psum col groups -> they run concurrently on the PE
                for st in range(4):
                    for b in range(BL):
                        nc.tensor.matmul(
                            ctx_ps[32 * b:32 * b + 32, :],
                            lhsT=wcol_sb[:, st, 32 * b:32 * b + 32],
                            rhs=p_an[b][:, st, :],
                            start=(p_sc == 0 and st == 0),
                            stop=(p_sc == NSC - 1 and st == 3),
                            tile_position=(0, 32 * b),
                        )
        pend = (w_sb, an_tiles, sc) if sc < NSC else None


def _make_in_maps(prev_hidden_state, annotations, W1, b1, W2):
    prev_hidden_state = np.asarray(prev_hidden_state, dtype=np.float32)
    annotations = np.asarray(annotations, dtype=np.float32)
    W1 = np.asarray(W1, dtype=np.float32)
    b1 = np.asarray(b1, dtype=np.float32)
    W2 = np.asarray(W2, dtype=np.float32)

    annN = annotations.astype(BF16)
    annT = np.ascontiguousarray(annotations.transpose(0, 2, 1)).astype(BF16)
    w1h = np.ascontiguousarray(W1[:H]).astype(BF16)
    w1a = np.ascontiguousarray(W1[H:]).astype(BF16)
    b1r = b1.reshape(1, H).astype(BF16)
    w2c = np.ascontiguousarray(np.tile(W2.reshape(H, 1), (1, 32))).astype(BF16)
    pvt = np.ascontiguousarray(prev_hidden_state.T).astype(BF16)  # [H, B]

    in_maps = []
    for c in range(NCORES):
        sl = slice(c * BL, (c + 1) * BL)
        in_maps.append(
            {
                "annT": np.ascontiguousarray(annT[sl]),
                "annN": np.ascontiguousarray(annN[sl]),
                "w1a": w1a,
                "w1h": w1h,
                "b1": b1r,
                "w2": w2c,
                "pvt": np.ascontiguousarray(pvt[:, sl]),
            }
        )
    return in_maps


def kernel(prev_hidden_state, annotations, W1, b1, W2, b2, **_unused):
    global _BUILT, LAST_RESULT
    from concourse import bass_utils

    # b2 shifts every score equally; softmax is shift-invariant -> ignored.
    in_maps = _make_in_maps(prev_hidden_state, annotations, W1, b1, W2)

    if _BUILT is None:
        _BUILT = _build_bass()
    nc = _BUILT

    trace = bool(int(os.environ.get("KERNEL_TRACE", "0")))
    if not trace:
        # the NTFF trace path needs antenv.axon_hooks, absent in this
        # client -- make sure an ambient BASS_TRACE can't select it
        os.environ.setdefault("BASS_NEVER_TRACE", "1")
    res = bass_utils.run_bass_kernel_spmd(
        nc, in_maps, core_ids=list(range(NCORES)), trace=trace
    )
    LAST_RESULT = res
    out = np.concatenate([r["out"] for r in res.results], axis=0)  # [B, A]
    return out[:, None, :].astype(np.float32)



# revision 8
# speedup vs baseline: 1.7179x; 1.7179x over previous
"""Bass/Tile TRN2 kernel for the attention module:

    pre    = prev_hidden @ W1[:H] + b1                    [B, H]
    hidden = tanh(pre[:, None, :] + ann @ W1[H:])         [B, S, H]
    score  = hidden @ W2 (+ b2; softmax-invariant, drop)  [B, S]
    alpha  = softmax(score, axis=1)
    ctx    = alpha @ ann                                  [B, 1, A]

B=32, S=4096, A=H=512. Sharding: data-parallel over batch, 4 batches per
core on 8 cores. Single pass over S per batch with an unnormalized
online softmax (scores are bounded: |score| <= sum|W2|+|b2| ~ 11.4, so
exp never overflows in fp32 and no running-max is needed):

    w_s = exp(score_s);  Z = sum w_s;  ctx = (sum w_s * ann_s) / Z

Layouts: the s-dim matmul (ann @ W1a) contracts over the feature dim a,
so it needs ann with a on SBUF partitions (annT); the context matmul
contracts over s, so it needs natural ann. Host supplies both layouts in
bf16 (same total HBM bytes as one fp32 copy).
"""

import os

import numpy as np
import ml_dtypes

B = 32
S = 4096
A = 512
H = 512
NCORES = 8
BL = B // NCORES  # 4 batches per core
SC = 512          # s-chunk processed per inner iteration
NSC = S // SC     # 8

BF16 = ml_dtypes.bfloat16
F8 = ml_dtypes.float8_e4m3  # maps to mybir.dt.float8e4 (TRN fp8 e4m3)
W1A_SCALE = 32.0  # W1a entries ~U(+-1/32); prescale into e4m3's normal range

_BUILT = None       # (nc,) cache — Bass module is reusable across calls
LAST_RESULT = None  # last BassKernelResults, for test harness introspection

LDW_DEDUP = False  # dropping LDWs breaks LDW<->MM pairing (verified wrong results)

# Stage selection for HW attribution profiling (all on for the real kernel)
STAGES = {"dma", "step2", "tanh", "score", "exp", "transpose", "ctx"}


def _build_bass(loop_n=None):
    """Build the Bass module. loop_n wraps the main s-loop in a For_i
    executed loop_n times — a timing amplifier (outputs then meaningless);
    loop_n=None builds the real single-pass kernel."""
    from contextlib import ExitStack, nullcontext

    import concourse.bass as bass
    import concourse.tile as tile
    from concourse import bacc, mybir
    from concourse.masks import make_identity

    bf16 = mybir.dt.bfloat16
    f8 = mybir.dt.float8e4
    f32 = mybir.dt.float32
    Tanh = mybir.ActivationFunctionType.Tanh
    Exp = mybir.ActivationFunctionType.Exp

    nc = bacc.Bacc()

    annT_d = nc.dram_tensor("annT", [BL, A, S], f8, kind="ExternalInput")
    annN_d = nc.dram_tensor("annN", [BL, S, A], bf16, kind="ExternalInput")
    w1a_d = nc.dram_tensor("w1a", [A, H], f8, kind="ExternalInput")
    w1h_d = nc.dram_tensor("w1h", [H, H], bf16, kind="ExternalInput")
    b1_d = nc.dram_tensor("b1", [1, H], bf16, kind="ExternalInput")
    w2_d = nc.dram_tensor("w2", [H, 32], bf16, kind="ExternalInput")
    pvt_d = nc.dram_tensor("pvt", [H, BL], bf16, kind="ExternalInput")
    out_d = nc.dram_tensor("out", [BL, A], f32, kind="ExternalOutput")

    with tile.TileContext(nc) as tc, ExitStack() as ctx:
        singles = ctx.enter_context(tc.tile_pool(name="singles", bufs=1))
        annt_pool = ctx.enter_context(tc.tile_pool(name="annt", bufs=4))
        annn_pool = ctx.enter_context(tc.tile_pool(name="annn", bufs=3))
        th_pool = ctx.enter_context(tc.tile_pool(name="thp", bufs=3))
        w_pool = ctx.enter_context(tc.tile_pool(name="wp", bufs=3))
        psum2 = ctx.enter_context(
            tc.tile_pool(name="psum2", bufs=1, space="PSUM")
        )
        psum_wc = ctx.enter_context(
            tc.tile_pool(name="psumwc", bufs=2, space="PSUM")
        )
        psum1 = ctx.enter_context(
            tc.tile_pool(name="psum1", bufs=1, space="PSUM")
        )

        # ---- constants / weights in SBUF ----
        ident = singles.tile([128, 128], bf16)
        make_identity(nc, ident)

        w1a_sb = singles.tile([128, 4, H], f8)  # (a%128, a//128, h)
        nc.sync.dma_start(
            out=w1a_sb, in_=w1a_d[:, :].rearrange("(ac p) h -> p ac h", p=128)
        )
        w1h_sb = singles.tile([128, 4, H], bf16)  # (hin%128, hin//128, h)
        nc.sync.dma_start(
            out=w1h_sb, in_=w1h_d[:, :].rearrange("(kc p) h -> p kc h", p=128)
        )
        b1_sb = singles.tile([1, H], bf16)
        nc.sync.dma_start(out=b1_sb, in_=b1_d[:, :])
        # W2 replicated x32 so score matmuls write a full 32-row col group
        w2_sb = singles.tile([128, 4, 32], bf16)  # (h%128, h//128, rep)
        nc.sync.dma_start(
            out=w2_sb, in_=w2_d[:, :].rearrange("(hc p) r -> p hc r", p=128)
        )
        pvt_sb = singles.tile([128, 4, BL], bf16)  # (hin%128, hin//128, b)
        nc.sync.dma_start(
            out=pvt_sb, in_=pvt_d[:, :].rearrange("(kc p) b -> p kc b", p=128)
        )
        ones_sb = singles.tile([1, BL], bf16)
        nc.vector.memset(ones_sb, 1.0)

        # ---- pre2T[h, b] = (prev @ W1h).T + b1 broadcast, in PSUM ----
        pre_ps = psum2.tile([128, 4, BL], f32, tag="score")
        for hc in range(4):
            for kc in range(4):
                nc.tensor.matmul(
                    pre_ps[:, hc, :],
                    lhsT=w1h_sb[:, kc, hc * 128:(hc + 1) * 128],
                    rhs=pvt_sb[:, kc, :],
                    start=(kc == 0),
                    stop=False,
                )
            # b1 contribution: rank-1 with ones row (K=1)
            nc.tensor.matmul(
                pre_ps[:, hc, :],
                lhsT=b1_sb[:, hc * 128:(hc + 1) * 128],
                rhs=ones_sb[:, :],
                start=False,
                stop=True,
            )
        pre_sb = singles.tile([128, 4, BL], f32)
        nc.scalar.copy(out=pre_sb, in_=pre_ps)

        # ---- main streaming loop over s-chunks ----
        z_sb = singles.tile([128, NSC], f32)
        ctx_ps = psum1.tile([128, A], f32, tag="ctx")

        outer = (
            tc.For_i(0, loop_n, 1) if loop_n is not None else nullcontext()
        )
        with outer:
            _main_body(
                nc, tc, mybir,
                annT_d, annN_d, w1a_sb, w2_sb, pre_sb, ident,
                annt_pool, annn_pool, th_pool, w_pool, psum2, psum_wc,
                z_sb, ctx_ps,
            )

        # ---- normalize and store ----
        out_sb = singles.tile([128, A], f32)
        if "exp" in STAGES and "ctx" in STAGES:
            z_tot = singles.tile([128, 1], f32)
            nc.vector.reduce_sum(
                out=z_tot, in_=z_sb, axis=mybir.AxisListType.X
            )
            z_rec = singles.tile([128, 1], f32)
            nc.vector.reciprocal(out=z_rec, in_=z_tot)
            nc.vector.tensor_scalar_mul(out_sb, ctx_ps[:, :], z_rec)
        else:
            nc.vector.memset(out_sb, 0.0)
        nc.sync.dma_start(out=out_d[:, :], in_=out_sb[0:128:32, :])

    if LDW_DEDUP:
        _dedup_ldweights(nc, mybir)
    nc.finalize()
    return nc


def _dedup_ldweights(nc, mybir):
    """Drop InstLdweights whose weights AP is identical to the previous
    (kept) InstLdweights with no different load in between; waits/updates
    are spliced onto the following instruction."""
    for f in nc.m.functions:
        for blk in f.blocks:
            insts = list(blk.instructions)
            keep = []
            last_key = None
            pending_sync = None
            for inst in insts:
                tn = type(inst).__name__
                if tn == "InstLdweights":
                    key = str(inst.ins[0])
                    if key == last_key:
                        si = inst.sync_info
                        if si is not None and (si.on_wait or si.on_update):
                            if pending_sync is None:
                                pending_sync = ([], [])
                            pending_sync[0].extend(si.on_wait)
                            pending_sync[1].extend(si.on_update)
                        continue  # drop it
                    last_key = key
                elif tn == "InstMatmult":
                    pass  # matmuls don't invalidate loaded weights
                else:
                    pass  # other-engine instrs in the block don't touch PE
                if pending_sync is not None:
                    si = inst.sync_info
                    ow = list(pending_sync[0])
                    ou = list(pending_sync[1])
                    if si is not None:
                        ow += list(si.on_wait)
                        ou += list(si.on_update)
                    inst.sync_info = mybir.SyncInfo(on_wait=ow, on_update=ou)
                    pending_sync = None
                keep.append(inst)
            if len(keep) != len(insts):
                blk.instructions = keep


def _main_body(
    nc, tc, mybir,
    annT_d, annN_d, w1a_sb, w2_sb, pre_sb, ident,
    annt_pool, annn_pool, th_pool, w_pool, psum2, psum_wc,
    z_sb, ctx_ps,
):
    bf16 = mybir.dt.bfloat16
    f8 = mybir.dt.float8e4
    f32 = mybir.dt.float32
    Tanh = mybir.ActivationFunctionType.Tanh
    Exp = mybir.ActivationFunctionType.Exp
    DR = mybir.MatmulPerfMode.DoubleRow

    # Batch-inner matmul ordering: 4 consecutive MMs share the stationary
    # weight block, and the transpose+ctx tail of chunk sc-1 is deferred so
    # its exp/DVE deps are resolved before the PE reaches it. The ann@W1a
    # matmul runs in fp8 e4m3 DoubleRow mode (K=256 per MM, half cycles/row).
    pend = None
    for sc in range(NSC + 1):
        if sc < NSC:
            score_ps = psum2.tile([128, SC], f32, tag="score")
            at_tiles, an_tiles, th_tiles, thp_tiles = [], [], [], []
            for b in range(BL):
                at_sb = annt_pool.tile([128, 4, SC], f8, tag=f"at{b}")
                if "dma" in STAGES:
                    nc.sync.dma_start(
                        out=at_sb,
                        in_=annT_d[b, :, sc * SC:(sc + 1) * SC].rearrange(
                            "(ac p) s -> p ac s", p=128
                        ),
                    )
                else:
                    nc.vector.memset(at_sb[:, 0, 0:1], 0.5)
                at_tiles.append(at_sb)
                an_sb = annn_pool.tile([128, 4, A], bf16, tag=f"an{b}")
                if "dma" in STAGES:
                    nc.sync.dma_start(
                        out=an_sb,
                        in_=annN_d[b, sc * SC:(sc + 1) * SC, :].rearrange(
                            "(sb p) a -> p sb a", p=128
                        ),
                    )
                else:
                    nc.vector.memset(an_sb[:, 0, 0:1], 0.5)
                an_tiles.append(an_sb)
                th_sb = th_pool.tile([128, 4, SC], bf16, tag=f"th{b}")
                if "step2" not in STAGES or "tanh" not in STAGES:
                    nc.vector.memset(th_sb[:, 0, 0:1], 0.5)
                th_tiles.append(th_sb)
                thp = psum2.tile([128, SC], f32, tag=f"thp{b}")
                thp_tiles.append(thp)

            if "step2" in STAGES:
                for hc in range(4):
                    for kh in range(2):
                        for b in range(BL):
                            nc.tensor.matmul(
                                thp_tiles[b][:, :],
                                lhsT=w1a_sb[
                                    :, 2 * kh:2 * kh + 2,
                                    hc * 128:(hc + 1) * 128,
                                ],
                                rhs=at_tiles[b][:, 2 * kh:2 * kh + 2, :],
                                start=(kh == 0),
                                stop=(kh == 1),
                                perf_mode=DR,
                            )
                    if "tanh" in STAGES:
                        for b in range(BL):
                            nc.scalar.activation(
                                out=th_tiles[b][:, hc, :],
                                in_=thp_tiles[b][:, :],
                                func=Tanh,
                                bias=pre_sb[:, hc, b:b + 1],
                                scale=1.0 / W1A_SCALE,
                            )
            else:
                for b in range(BL):
                    nc.vector.memset(thp_tiles[b][:, 0:1], 0.5)

            if "score" in STAGES:
                for hc in range(4):
                    for b in range(BL):
                        nc.tensor.matmul(
                            score_ps[32 * b:32 * b + 32, :],
                            lhsT=w2_sb[:, hc, :],
                            rhs=th_tiles[b][:, hc, :],
                            start=(hc == 0),
                            stop=(hc == 3),
                            tile_position=(0, 32 * b),
                        )
            else:
                nc.vector.memset(score_ps[:, 0:1], 0.5)

            w_sb = w_pool.tile([128, SC], bf16, tag="w")
            if "exp" in STAGES:
                nc.scalar.activation(
                    out=w_sb,
                    in_=score_ps[:, :],
                    func=Exp,
                    accum_out=z_sb[:, sc:sc + 1],
                )
            else:
                nc.vector.memset(w_sb[:, 0:1], 0.5)
        else:
            w_sb = None
            an_tiles = None

        if pend is not None:
            p_w, p_an, p_sc = pend
            wcol_sb = w_pool.tile([128, 4, 128], bf16, tag="wcol")
            if "transpose" in STAGES:
                for st in range(4):
                    wc_ps = psum_wc.tile([128, 128], bf16, tag="wc")
                    nc.tensor.transpose(
                        wc_ps[:, :], p_w[:, st * 128:(st + 1) * 128],
                        ident[:, :],
                    )
                    nc.vector.tensor_copy(
                        out=wcol_sb[:, st, :], in_=wc_ps[:, :]
                    )
            else:
                nc.vector.memset(wcol_sb[:, 0, 0:1], 0.5)
            if "ctx" in STAGES:
                # st outer / b inner: consecutive MMs target disjoint
                # psum col groups -> they run concurrently on the PE
                for st in range(4):
                    for b in range(BL):
                        nc.tensor.matmul(
                            ctx_ps[32 * b:32 * b + 32, :],
                            lhsT=wcol_sb[:, st, 32 * b:32 * b + 32],
                            rhs=p_an[b][:, st, :],
                            start=(p_sc == 0 and st == 0),
                            stop=(p_sc == NSC - 1 and st == 3),
                            tile_position=(0, 32 * b),
                        )
        pend = (w_sb, an_tiles, sc) if sc < NSC else None


def _make_in_maps(prev_hidden_state, annotations, W1, b1, W2):
    prev_hidden_state = np.asarray(prev_hidden_state, dtype=np.float32)
    annotations = np.asarray(annotations, dtype=np.float32)
    W1 = np.asarray(W1, dtype=np.float32)
    b1 = np.asarray(b1, dtype=np.float32)
    W2 = np.asarray(W2, dtype=np.float32)

    annN = annotations.astype(BF16)
    annT = np.ascontiguousarray(annotations.transpose(0, 2, 1)).astype(F8)
    w1h = np.ascontiguousarray(W1[:H]).astype(BF16)
    w1a = np.ascontiguousarray(W1[H:] * W1A_SCALE).astype(F8)
    b1r = b1.reshape(1, H).astype(BF16)
    w2c = np.ascontiguousarray(np.tile(W2.reshape(H, 1), (1, 32))).astype(BF16)
    pvt = np.ascontiguousarray(prev_hidden_state.T).astype(BF16)  # [H, B]

    in_maps = []
    for c in range(NCORES):
        sl = slice(c * BL, (c + 1) * BL)
        in_maps.append(
            {
                "annT": np.ascontiguousarray(annT[sl]),
                "annN": np.ascontiguousarray(annN[sl]),
                "w1a": w1a,
                "w1h": w1h,
                "b1": b1r,
                "w2": w2c,
                "pvt": np.ascontiguousarray(pvt[:, sl]),
            }
        )
    return in_maps


def kernel(prev_hidden_state, annotations, W1, b1, W2, b2, **_unused):
    global _BUILT, LAST_RESULT
    from concourse import bass_utils

    # b2 shifts every score equally; softmax is shift-invariant -> ignored.
    in_maps = _make_in_maps(prev_hidden_state, annotations, W1, b1, W2)

    if _BUILT is None:
        _BUILT = _build_bass()
    nc = _BUILT

    trace = bool(int(os.environ.get("KERNEL_TRACE", "0")))
    if not trace:
        # the NTFF trace path needs antenv.axon_hooks, absent in this
        # client -- make sure an ambient BASS_TRACE can't select it
        os.environ.setdefault("BASS_NEVER_TRACE", "1")
    res = bass_utils.run_bass_kernel_spmd(
        nc, in_maps, core_ids=list(range(NCORES)), trace=trace
    )
    LAST_RESULT = res
    out = np.concatenate([r["out"] for r in res.results], axis=0)  # [B, A]
    return out[:, None, :].astype(np.float32)



# revision 27
# speedup vs baseline: 1.7463x; 1.0165x over previous
"""Bass/Tile TRN2 kernel for the attention module:

    pre    = prev_hidden @ W1[:H] + b1                    [B, H]
    hidden = tanh(pre[:, None, :] + ann @ W1[H:])         [B, S, H]
    score  = hidden @ W2 (+ b2; softmax-invariant, drop)  [B, S]
    alpha  = softmax(score, axis=1)
    ctx    = alpha @ ann                                  [B, 1, A]

B=32, S=4096, A=H=512. Sharding: data-parallel over batch, 4 batches per
core on 8 cores. Single pass over S per batch with an unnormalized
online softmax (scores are bounded: |score| <= sum|W2|+|b2| ~ 11.4, so
exp never overflows in fp32 and no running-max is needed):

    w_s = exp(score_s);  Z = sum w_s;  ctx = (sum w_s * ann_s) / Z

Layouts: the s-dim matmul (ann @ W1a) contracts over the feature dim a,
so it needs ann with a on SBUF partitions (annT, fp8 e4m3); the context
matmul contracts over s, so it needs natural ann (annN, bf16). The
ann @ W1a matmul runs in fp8 DoubleRow mode (K=256 per MM, 0.5
cycles/row): W1a is host-prescaled x32 into e4m3's normal range and the
1/32 is folded into the tanh activation's scale.

Schedule: s-chunks of 512 are processed in PAIRS so each tanh covers
1024 elements per partition (fewer Activation-engine calls). Within a
pair the in-order PE interleaves step2 (fp8 DR), the previous hc's
score MMs, and the previous pair's ctx MMs as filler, so it never
head-blocks on the Activation engine's tanh stream. The softmax-weight
transpose (w -> s-on-partitions) runs on the DMA xbar
(dma_start_transpose from the Act queue), not the PE. annT DMAs are
issued one pair ahead of use; annN (first needed by the deferred ctx
tail one pair later) trails in the same queue.
"""

import os

import numpy as np
import ml_dtypes

B = 32
S = 4096
A = 512
H = 512
NCORES = 8
BL = B // NCORES  # 4 batches per core
SC = 512          # s-chunk per matmul moving operand
NSC = S // SC     # 8
NPAIR = NSC // 2  # chunk pairs per core

BF16 = ml_dtypes.bfloat16
F8 = ml_dtypes.float8_e4m3  # maps to mybir.dt.float8e4 (TRN fp8 e4m3)
W1A_SCALE = 32.0  # W1a entries ~U(+-1/32); prescale into e4m3's normal range

_BUILT = None       # (nc,) cache — Bass module is reusable across calls
LAST_RESULT = None  # last BassKernelResults, for test harness introspection


def _build_bass(loop_n=None):
    """Build the Bass module. loop_n wraps the main pair-loop in a For_i
    executed loop_n times — a timing amplifier (outputs then meaningless);
    loop_n=None builds the real single-pass kernel."""
    from contextlib import ExitStack, nullcontext

    import concourse.bass as bass
    import concourse.tile as tile
    from concourse import bacc, mybir
    from concourse.masks import make_identity

    bf16 = mybir.dt.bfloat16
    f8 = mybir.dt.float8e4
    f32 = mybir.dt.float32

    nc = bacc.Bacc()

    annT_d = nc.dram_tensor("annT", [BL, A, S], f8, kind="ExternalInput")
    annN_d = nc.dram_tensor("annN", [BL, S, A], bf16, kind="ExternalInput")
    w1a_d = nc.dram_tensor("w1a", [A, H], f8, kind="ExternalInput")
    w1h_d = nc.dram_tensor("w1h", [H, H], bf16, kind="ExternalInput")
    b1_d = nc.dram_tensor("b1", [1, H], bf16, kind="ExternalInput")
    w2_d = nc.dram_tensor("w2", [H, 32], bf16, kind="ExternalInput")
    pvt_d = nc.dram_tensor("pvt", [H, BL], bf16, kind="ExternalInput")
    out_d = nc.dram_tensor("out", [BL, A], f32, kind="ExternalOutput")

    with tile.TileContext(nc) as tc, ExitStack() as ctx:
        singles = ctx.enter_context(tc.tile_pool(name="singles", bufs=1))
        annt_pool = ctx.enter_context(tc.tile_pool(name="annt", bufs=2))
        annn_pool = ctx.enter_context(tc.tile_pool(name="annn", bufs=2))
        th_pool = ctx.enter_context(tc.tile_pool(name="thp", bufs=2))
        w_pool = ctx.enter_context(tc.tile_pool(name="wp", bufs=2))
        psum_th = ctx.enter_context(
            tc.tile_pool(name="psumth", bufs=1, space="PSUM")
        )
        psum_sc = ctx.enter_context(
            tc.tile_pool(name="psumsc", bufs=1, space="PSUM")
        )
        psum_wc = ctx.enter_context(
            tc.tile_pool(name="psumwc", bufs=1, space="PSUM")
        )
        psum1 = ctx.enter_context(
            tc.tile_pool(name="psum1", bufs=1, space="PSUM")
        )

        # ---- weights / constants into SBUF ----
        # DMA queue order matters: w1h/pvt/b1 first so the pre matmul (the
        # Activation engine's gating dependency for every tanh) lands as
        # early as possible; then w1a + the first pair's annT for the PE's
        # first step2 slots.
        w1h_sb = singles.tile([128, 4, H], bf16)  # (hin%128, hin//128, h)
        nc.sync.dma_start(
            out=w1h_sb, in_=w1h_d[:, :].rearrange("(kc p) h -> p kc h", p=128)
        )
        pvt_sb = singles.tile([128, 4, BL], bf16)  # (hin%128, hin//128, b)
        nc.sync.dma_start(
            out=pvt_sb, in_=pvt_d[:, :].rearrange("(kc p) b -> p kc b", p=128)
        )
        b1_sb = singles.tile([1, H], bf16)
        nc.sync.dma_start(out=b1_sb, in_=b1_d[:, :])

        w1a_sb = singles.tile([128, 4, H], f8)  # (a%128, a//128, h)
        nc.sync.dma_start(
            out=w1a_sb, in_=w1a_d[:, :].rearrange("(ac p) h -> p ac h", p=128)
        )

        at_tiles = {}  # (sp, sub, b) -> tile, DMA-issued one pair ahead

        def issue_at(sp, bs):
            for b in bs:
                for sub in range(2):
                    sc = 2 * sp + sub
                    t = annt_pool.tile([128, 4, SC], f8, tag=f"at{sub}{b}")
                    nc.sync.dma_start(
                        out=t,
                        in_=annT_d[b, :, sc * SC:(sc + 1) * SC].rearrange(
                            "(ac p) s -> p ac s", p=128
                        ),
                    )
                    at_tiles[(sp, sub, b)] = t

        issue_at(0, (0, 1))

        # W2 replicated x32 so score matmuls write a full 32-row col group
        w2_sb = singles.tile([128, 4, 32], bf16)  # (h%128, h//128, rep)
        nc.sync.dma_start(
            out=w2_sb, in_=w2_d[:, :].rearrange("(hc p) r -> p hc r", p=128)
        )

        issue_at(0, (2, 3))

        ones_sb = singles.tile([1, BL], bf16)
        nc.vector.memset(ones_sb, 1.0)
        ident = singles.tile([128, 128], bf16)
        make_identity(nc, ident)

        pre_sb = singles.tile([128, 4, BL], f32)

        def emit_pre():
            # pre2T[h, b] = (prev @ W1h).T + b1 broadcast.
            pre_ps = psum_sc.tile([128, 2, SC], f32, tag="score")
            for hc in range(4):
                for kc in range(4):
                    nc.tensor.matmul(
                        pre_ps[:, 0, hc * BL:(hc + 1) * BL],
                        lhsT=w1h_sb[:, kc, hc * 128:(hc + 1) * 128],
                        rhs=pvt_sb[:, kc, :],
                        start=(kc == 0),
                        stop=False,
                    )
                # b1 contribution: rank-1 with ones row (K=1)
                nc.tensor.matmul(
                    pre_ps[:, 0, hc * BL:(hc + 1) * BL],
                    lhsT=b1_sb[:, hc * 128:(hc + 1) * 128],
                    rhs=ones_sb[:, :],
                    start=False,
                    stop=True,
                )
            nc.scalar.copy(
                out=pre_sb,
                in_=pre_ps[:, 0, 0:4 * BL].rearrange(
                    "p (hc b) -> p hc b", b=BL
                ),
            )

        z_sb = singles.tile([128, 2 * NPAIR], f32)
        z_tot_sb = singles.tile([128, 1], f32)
        z_rec_sb = singles.tile([128, 1], f32)
        ctx_ps = psum1.tile([128, A], f32, tag="ctx")

        outer = (
            tc.For_i(0, loop_n, 1) if loop_n is not None else nullcontext()
        )
        with outer:
            _main_body(
                nc, tc, mybir,
                annN_d, w1a_sb, w2_sb, pre_sb, ident,
                at_tiles, issue_at, emit_pre,
                annn_pool, th_pool, w_pool, psum_th, psum_sc, psum_wc,
                z_sb, z_tot_sb, z_rec_sb, ctx_ps,
                amplified=loop_n is not None,
            )

        # ---- normalize and store ----
        out_sb = singles.tile([128, A], f32)
        nc.vector.tensor_scalar_mul(out_sb, ctx_ps[:, :], z_rec_sb)
        nc.sync.dma_start(out=out_d[:, :], in_=out_sb[0:128:32, :])

    nc.finalize()
    return nc


def _main_body(
    nc, tc, mybir,
    annN_d, w1a_sb, w2_sb, pre_sb, ident,
    at_tiles, issue_at, emit_pre,
    annn_pool, th_pool, w_pool, psum_th, psum_sc, psum_wc,
    z_sb, z_tot_sb, z_rec_sb, ctx_ps,
    amplified=False,
):
    bf16 = mybir.dt.bfloat16
    f8 = mybir.dt.float8e4
    f32 = mybir.dt.float32
    Tanh = mybir.ActivationFunctionType.Tanh
    Exp = mybir.ActivationFunctionType.Exp
    DR = mybir.MatmulPerfMode.DoubleRow

    # The w-transpose + ctx MMs of pair sp-1 are deferred into iteration sp
    # and SPREAD through its hc/b slots as PE filler, so the in-order PE
    # never head-blocks on the Activation engine's tanh stream.
    emit_pre()
    pend = None
    for sp in range(NPAIR + 1):
        if sp < NPAIR:
            if amplified:
                if sp > 0:
                    issue_at(sp, range(BL))  # no prefetch under For_i
            elif sp + 1 < NPAIR:
                issue_at(sp + 1, range(BL))  # prefetch next pair's annT
            an_tiles = []  # an_tiles[b][sub]
            for b in range(BL):
                subs = []
                for sub in range(2):
                    sc = 2 * sp + sub
                    t = annn_pool.tile([128, 4, A], bf16, tag=f"an{sub}{b}")
                    nc.sync.dma_start(
                        out=t,
                        in_=annN_d[b, sc * SC:(sc + 1) * SC, :].rearrange(
                            "(sb p) a -> p sb a", p=128
                        ),
                    )
                    subs.append(t)
                an_tiles.append(subs)

            th_tiles = [
                th_pool.tile(
                    [128, 4, 2, SC], bf16, tag=f"th{b}", name=f"th{b}"
                )
                for b in range(BL)
            ]
            score_ps = psum_sc.tile([128, 2, SC], f32, tag="score")

        if pend is not None:
            p_w, p_an, p_sp = pend
            wcol_sb = w_pool.tile([128, 8, 128], bf16, tag="wcol")
            ctx_jobs = [(st, b) for st in range(8) for b in range(BL)]
        else:
            ctx_jobs = []

        def emit_transpose(st):
            # PE transpose of one 128-col block of the previous pair's w,
            # PSUM-staged, copied to SBUF by the (otherwise idle) DVE.
            wc_ps = psum_wc.tile([128, 128], bf16, tag="wc")
            nc.tensor.transpose(
                wc_ps[:, :],
                p_w[:, st // 4, (st % 4) * 128:(st % 4 + 1) * 128],
                ident[:, :],
            )
            nc.vector.tensor_copy(out=wcol_sb[:, st, :], in_=wc_ps[:, :])

        def emit_ctx(n):
            for _ in range(n):
                if not ctx_jobs:
                    return
                st, b = ctx_jobs.pop(0)
                nc.tensor.matmul(
                    ctx_ps[32 * b:32 * b + 32, :],
                    lhsT=wcol_sb[:, st, 32 * b:32 * b + 32],
                    rhs=p_an[b][st // 4][:, st % 4, :],
                    start=(p_sp == 0 and st == 0),
                    stop=(p_sp == NPAIR - 1 and st == 7),
                    tile_position=(0, 32 * b),
                )

        if sp < NPAIR:

            def emit_score(hc, b):
                for sub in range(2):
                    nc.tensor.matmul(
                        score_ps[32 * b:32 * b + 32, sub, :],
                        lhsT=w2_sb[:, hc, :],
                        rhs=th_tiles[b][:, hc, sub, :],
                        start=(hc == 0),
                        stop=(hc == 3),
                        tile_position=(0, 32 * b),
                    )

            slot = 0
            for hc in range(4):
                for b in range(BL):
                    thp = psum_th.tile([128, 2, SC], f32, tag=f"thp{b % 2}")
                    for sub in range(2):
                        for kh in range(2):
                            nc.tensor.matmul(
                                thp[:, sub, :],
                                lhsT=w1a_sb[
                                    :, 2 * kh:2 * kh + 2,
                                    hc * 128:(hc + 1) * 128,
                                ],
                                rhs=at_tiles[(sp, sub, b)][
                                    :, 2 * kh:2 * kh + 2, :
                                ],
                                start=(kh == 0),
                                stop=(kh == 1),
                                perf_mode=DR,
                            )
                    nc.scalar.activation(
                        out=th_tiles[b][:, hc, :, :],
                        in_=thp[:, :, :],
                        func=Tanh,
                        bias=pre_sb[:, hc, b:b + 1],
                        scale=1.0 / W1A_SCALE,
                    )
                    if hc > 0:
                        emit_score(hc - 1, b)
                    if pend is not None and slot < 8:
                        emit_transpose(slot)
                    if slot >= 3:
                        emit_ctx(3)
                    slot += 1
            for b in range(BL):
                emit_score(3, b)
                emit_ctx(2)
            emit_ctx(len(ctx_jobs))

            # exp per sub, accumulating Z into per-(pair,sub) columns
            w_sb = w_pool.tile([128, 2, SC], bf16, tag="w")
            for sub in range(2):
                nc.scalar.activation(
                    out=w_sb[:, sub, :],
                    in_=score_ps[:, sub, :],
                    func=Exp,
                    accum_out=z_sb[:, 2 * sp + sub:2 * sp + sub + 1],
                )
            pend = (w_sb, an_tiles, sp)
        else:
            # last pair: reduce Z while the PE drains the deferred
            # transposes and ctx MMs (interleaved to hide the single
            # wc-bank PE<->DVE ping-pong; st 0-3 depend only on exp(sub0),
            # so they overlap exp(sub1) on the Act engine)
            nc.vector.reduce_sum(
                out=z_tot_sb, in_=z_sb, axis=mybir.AxisListType.X
            )
            nc.vector.reciprocal(out=z_rec_sb, in_=z_tot_sb)
            emit_transpose(0)
            emit_transpose(1)
            for st in range(8):
                if st + 2 < 8:
                    emit_transpose(st + 2)
                emit_ctx(4)
            pend = None


def _make_in_maps(prev_hidden_state, annotations, W1, b1, W2):
    prev_hidden_state = np.asarray(prev_hidden_state, dtype=np.float32)
    annotations = np.asarray(annotations, dtype=np.float32)
    W1 = np.asarray(W1, dtype=np.float32)
    b1 = np.asarray(b1, dtype=np.float32)
    W2 = np.asarray(W2, dtype=np.float32)

    annN = annotations.astype(BF16)
    annT = np.ascontiguousarray(annotations.transpose(0, 2, 1)).astype(F8)
    w1h = np.ascontiguousarray(W1[:H]).astype(BF16)
    w1a = np.ascontiguousarray(W1[H:] * W1A_SCALE).astype(F8)
    b1r = b1.reshape(1, H).astype(BF16)
    w2c = np.ascontiguousarray(np.tile(W2.reshape(H, 1), (1, 32))).astype(BF16)
    pvt = np.ascontiguousarray(prev_hidden_state.T).astype(BF16)  # [H, B]

    in_maps = []
    for c in range(NCORES):
        sl = slice(c * BL, (c + 1) * BL)
        in_maps.append(
            {
                "annT": np.ascontiguousarray(annT[sl]),
                "annN": np.ascontiguousarray(annN[sl]),
                "w1a": w1a,
                "w1h": w1h,
                "b1": b1r,
                "w2": w2c,
                "pvt": np.ascontiguousarray(pvt[:, sl]),
            }
        )
    return in_maps


def kernel(prev_hidden_state, annotations, W1, b1, W2, b2, **_unused):
    global _BUILT, LAST_RESULT
    from concourse import bass_utils

    # b2 shifts every score equally; softmax is shift-invariant -> ignored.
    in_maps = _make_in_maps(prev_hidden_state, annotations, W1, b1, W2)

    if _BUILT is None:
        _BUILT = _build_bass()
    nc = _BUILT

    trace = bool(int(os.environ.get("KERNEL_TRACE", "0")))
    if not trace:
        # the NTFF trace path needs antenv.axon_hooks, absent in this
        # client -- make sure an ambient BASS_TRACE can't select it
        os.environ.setdefault("BASS_NEVER_TRACE", "1")
    res = bass_utils.run_bass_kernel_spmd(
        nc, in_maps, core_ids=list(range(NCORES)), trace=trace
    )
    LAST_RESULT = res
    out = np.concatenate([r["out"] for r in res.results], axis=0)  # [B, A]
    return out[:, None, :].astype(np.float32)


# revision 49
# speedup vs baseline: 2.1685x; 1.2417x over previous
"""Bass/Tile TRN2 kernel for the attention module:

    pre    = prev_hidden @ W1[:H] + b1                    [B, H]
    hidden = tanh(pre[:, None, :] + ann @ W1[H:])         [B, S, H]
    score  = hidden @ W2 (+ b2; softmax-invariant, drop)  [B, S]
    alpha  = softmax(score, axis=1)
    ctx    = alpha @ ann                                  [B, 1, A]

B=32, S=4096, A=H=512. Sharding: data-parallel over batch, 4 batches per
core on 8 cores. Single pass over S per batch with an unnormalized
online softmax (scores are bounded: |score| <= sum|W2|+|b2| ~ 11.4, so
exp never overflows in fp32 and no running-max is needed):

    w_s = exp(score_s);  Z = sum w_s;  ctx = (sum w_s * ann_s) / Z

Layouts: the s-dim matmul (ann @ W1a) contracts over the feature dim a,
so it needs ann with a on SBUF partitions (annT, fp8 e4m3); the context
matmul contracts over s, so it needs natural ann (annN, bf16). The
ann @ W1a matmul runs in fp8 DoubleRow mode (K=256 per MM, 0.5
cycles/row): W1a is host-prescaled x32 into e4m3's normal range and the
1/32 is folded into the tanh activation's scale.

Schedule: s-chunks of 512 are processed in PAIRS so each tanh covers
1024 elements per partition (fewer Activation-engine calls). The score
is computed TRANSPOSED (s on partitions) by 1-column matmuls whose
stationary operand is the tanh tile and whose moving operand is the W2
vector, so exp directly yields softmax-weight columns in the layout the
ctx matmuls consume -- no weight transpose exists anywhere. The ctx
matmuls likewise make annN the stationary operand and stream one
w-column each (output free size 1), accumulating per-pair PSUM groups
that the otherwise-idle DVE folds into an SBUF accumulator (PSUM allows
only one open accumulation group per zero region, so kernel-lifetime
column groups are illegal). Z comes from a ones-vector contraction of
the weight columns plus a small selector matmul at the end. Within a
pair the in-order PE interleaves step2 (fp8 DR), the previous batch's
score groups, and the previous pair's ctx groups as filler, so it never
head-blocks on the Activation engine's tanh stream. annT DMAs are
issued one pair ahead of use; annN (first needed by the deferred ctx
tail one pair later) trails in the same queue.
"""

import os

import numpy as np
import ml_dtypes

B = 32
S = 4096
A = 512
H = 512
NCORES = 8
BL = B // NCORES  # 4 batches per core
SC = 512          # s-chunk per matmul moving operand
NSC = S // SC     # 8
NPAIR = NSC // 2  # chunk pairs per core

BF16 = ml_dtypes.bfloat16
F8 = ml_dtypes.float8_e4m3  # maps to mybir.dt.float8e4 (TRN fp8 e4m3)
W1A_SCALE = 32.0  # W1a entries ~U(+-1/32); prescale into e4m3's normal range

_BUILT = None       # (nc,) cache — Bass module is reusable across calls
LAST_RESULT = None  # last BassKernelResults, for test harness introspection


def _build_bass(loop_n=None):
    """Build the Bass module. loop_n wraps the main pair-loop in a For_i
    executed loop_n times — a timing amplifier (outputs then meaningless);
    loop_n=None builds the real single-pass kernel."""
    from contextlib import ExitStack, nullcontext

    import concourse.bass as bass
    import concourse.tile as tile
    from concourse import bacc, mybir
    from concourse.masks import make_identity

    bf16 = mybir.dt.bfloat16
    f8 = mybir.dt.float8e4
    f32 = mybir.dt.float32

    nc = bacc.Bacc()

    annT_d = nc.dram_tensor("annT", [BL, A, S], f8, kind="ExternalInput")
    annN_d = nc.dram_tensor("annN", [BL, S, A], bf16, kind="ExternalInput")
    w1a_d = nc.dram_tensor("w1a", [A, H], f8, kind="ExternalInput")
    w1h_d = nc.dram_tensor("w1h", [H, H], bf16, kind="ExternalInput")
    b1_d = nc.dram_tensor("b1", [1, H], bf16, kind="ExternalInput")
    w2_d = nc.dram_tensor("w2", [H, 1], bf16, kind="ExternalInput")
    # sel[k, b] = 1 if k % BL == b: partition-group selector for the Z
    # reduction (sums the per-(sub,st) column partials of batch b)
    sel_d = nc.dram_tensor("sel", [32, BL], f32, kind="ExternalInput")
    pvt_d = nc.dram_tensor("pvt", [H, BL], bf16, kind="ExternalInput")
    out_d = nc.dram_tensor("out", [BL, A], f32, kind="ExternalOutput")

    with tile.TileContext(nc) as tc, ExitStack() as ctx:
        singles = ctx.enter_context(tc.tile_pool(name="singles", bufs=1))
        annt_pool = ctx.enter_context(tc.tile_pool(name="annt", bufs=2))
        annn_pool = ctx.enter_context(tc.tile_pool(name="annn", bufs=2))
        th_pool = ctx.enter_context(tc.tile_pool(name="thp", bufs=2))
        w_pool = ctx.enter_context(tc.tile_pool(name="wp", bufs=2))
        psum_th = ctx.enter_context(
            tc.tile_pool(name="psumth", bufs=1, space="PSUM")
        )
        psum_sc = ctx.enter_context(
            tc.tile_pool(name="psumsc", bufs=1, space="PSUM")
        )
        psum_z = ctx.enter_context(
            tc.tile_pool(name="psumz", bufs=1, space="PSUM")
        )
        psum1 = ctx.enter_context(
            tc.tile_pool(name="psum1", bufs=1, space="PSUM")
        )

        # ---- weights / constants into SBUF ----
        # DMA queue order matters: w1h/pvt/b1 first so the pre matmul (the
        # Activation engine's gating dependency for every tanh) lands as
        # early as possible; then w1a + the first pair's annT for the PE's
        # first step2 slots.
        w1h_sb = singles.tile([128, 4, H], bf16)  # (hin%128, hin//128, h)
        nc.sync.dma_start(
            out=w1h_sb, in_=w1h_d[:, :].rearrange("(kc p) h -> p kc h", p=128)
        )
        pvt_sb = singles.tile([128, 4, BL], bf16)  # (hin%128, hin//128, b)
        nc.sync.dma_start(
            out=pvt_sb, in_=pvt_d[:, :].rearrange("(kc p) b -> p kc b", p=128)
        )
        b1_sb = singles.tile([1, H], bf16)
        nc.sync.dma_start(out=b1_sb, in_=b1_d[:, :])

        w1a_sb = singles.tile([128, 4, H], f8)  # (a%128, a//128, h)
        nc.sync.dma_start(
            out=w1a_sb, in_=w1a_d[:, :].rearrange("(ac p) h -> p ac h", p=128)
        )

        at_tiles = {}  # (sp, sub, b) -> tile, DMA-issued one pair ahead

        def issue_at(sp, bs):
            for b in bs:
                for sub in range(2):
                    sc = 2 * sp + sub
                    t = annt_pool.tile([128, 4, SC], f8, tag=f"at{sub}{b}")
                    nc.sync.dma_start(
                        out=t,
                        in_=annT_d[b, :, sc * SC:(sc + 1) * SC].rearrange(
                            "(ac p) s -> p ac s", p=128
                        ),
                    )
                    at_tiles[(sp, sub, b)] = t

        issue_at(0, (0, 1))

        w2_sb = singles.tile([128, 4, 1], bf16)  # (h%128, h//128, 1)
        nc.sync.dma_start(
            out=w2_sb, in_=w2_d[:, :].rearrange("(hc p) r -> p hc r", p=128)
        )
        sel_sb = singles.tile([32, BL], f32)
        nc.sync.dma_start(out=sel_sb, in_=sel_d[:, :])

        issue_at(0, (2, 3))

        ones_sb = singles.tile([1, BL], bf16)
        nc.vector.memset(ones_sb, 1.0)
        onesc_sb = singles.tile([128, 1], bf16)
        nc.vector.memset(onesc_sb, 1.0)
        onesr_sb = singles.tile([1, 128], f32)
        nc.vector.memset(onesr_sb, 1.0)
        identf = singles.tile([128, 128], f32)
        make_identity(nc, identf)

        pre_sb = singles.tile([128, 4, BL], f32)

        def emit_pre():
            # pre2T[h, b] = (prev @ W1h).T + b1 broadcast.
            pre_ps = psum_sc.tile([128, 32], f32, tag="score")
            for hc in range(4):
                for kc in range(4):
                    nc.tensor.matmul(
                        pre_ps[:, hc * BL:(hc + 1) * BL],
                        lhsT=w1h_sb[:, kc, hc * 128:(hc + 1) * 128],
                        rhs=pvt_sb[:, kc, :],
                        start=(kc == 0),
                        stop=False,
                    )
                # b1 contribution: rank-1 with ones row (K=1)
                nc.tensor.matmul(
                    pre_ps[:, hc * BL:(hc + 1) * BL],
                    lhsT=b1_sb[:, hc * 128:(hc + 1) * 128],
                    rhs=ones_sb[:, :],
                    start=False,
                    stop=True,
                )
            nc.scalar.copy(
                out=pre_sb,
                in_=pre_ps[:, 0:4 * BL].rearrange(
                    "p (hc b) -> p hc b", b=BL
                ),
            )

        ctx_acc = singles.tile([128, BL, 4], f32)  # (a%128, b, ac)
        nc.vector.memset(ctx_acc, 0.0)
        zp_acc = singles.tile([32, 1], f32)  # per-(sub,st,b) Z partials
        nc.vector.memset(zp_acc, 0.0)

        outer = (
            tc.For_i(0, loop_n, 1) if loop_n is not None else nullcontext()
        )
        with outer:
            _main_body(
                nc, tc, mybir,
                annN_d, w1a_sb, w2_sb, pre_sb,
                at_tiles, issue_at, emit_pre,
                annn_pool, th_pool, w_pool, psum_th, psum_sc, psum_z,
                psum1, onesc_sb, ctx_acc, zp_acc,
                amplified=loop_n is not None,
            )

        # ---- normalize and store ----
        # Z per batch: group-sum the [32,1] column partials via the selector
        # matmul, reciprocal, transpose to a row, broadcast over partitions,
        # then scale ctx and store.
        zb_ps = psum_sc.tile([BL, 1], f32, tag="score")
        nc.tensor.matmul(
            zb_ps[:, :], lhsT=sel_sb[:, :], rhs=zp_acc[:, :],
            start=True, stop=True,
        )
        zrec_sb = singles.tile([BL, 1], f32)
        nc.vector.reciprocal(out=zrec_sb, in_=zb_ps)
        zrt_ps = psum_sc.tile([1, BL], f32, tag="score")
        nc.tensor.transpose(
            zrt_ps[:, :], zrec_sb[:, :], identf[0:BL, 0:BL]
        )
        zrt_sb = singles.tile([1, BL], f32)
        nc.vector.tensor_copy(out=zrt_sb, in_=zrt_ps[:, :])
        bc_ps = psum_sc.tile([128, BL], f32, tag="score")
        nc.tensor.matmul(
            bc_ps[:, :], lhsT=onesr_sb[:, :], rhs=zrt_sb[:, :],
            start=True, stop=True,
        )
        out_sb = singles.tile([128, BL, 4], f32)
        for b in range(BL):
            nc.vector.tensor_scalar_mul(
                out_sb[:, b, :], ctx_acc[:, b, :], bc_ps[:, b:b + 1]
            )
        outt_ps = psum_sc.tile([4 * BL, 128], f32, tag="score")
        nc.tensor.transpose(
            outt_ps[:, :],
            out_sb[:, :, :].rearrange("p b ac -> p (b ac)"),
            identf[:, :],
        )
        outt_sb = singles.tile([4 * BL, 128], f32)
        nc.vector.tensor_copy(out=outt_sb, in_=outt_ps[:, :])
        nc.sync.dma_start(
            out=out_d[:, :].rearrange("b (ac p) -> (b ac) p", p=128),
            in_=outt_sb,
        )

    nc.finalize()
    return nc


def _main_body(
    nc, tc, mybir,
    annN_d, w1a_sb, w2_sb, pre_sb,
    at_tiles, issue_at, emit_pre,
    annn_pool, th_pool, w_pool, psum_th, psum_sc, psum_z,
    psum1, onesc_sb, ctx_acc, zp_acc,
    amplified=False,
):
    bf16 = mybir.dt.bfloat16
    f8 = mybir.dt.float8e4
    f32 = mybir.dt.float32
    Tanh = mybir.ActivationFunctionType.Tanh
    Exp = mybir.ActivationFunctionType.Exp
    DR = mybir.MatmulPerfMode.DoubleRow

    # Score is computed TRANSPOSED (s on partitions): scoreT[s, col] with
    # col = sub*16 + st*4 + b, via tiny 1-column matmuls whose STATIONARY
    # operand is the th tile (lhsT) and whose moving operand is the W2
    # vector. exp then produces the softmax weights already in the layout
    # the ctx matmuls need as moving columns, so no w transpose exists at
    # all. The ctx matmuls likewise make annN the stationary operand and
    # stream one w column (out free size 1). The ctx MMs of pair sp-1 are
    # deferred into iteration sp and spread through its slots as filler.
    emit_pre()
    pend = None
    for sp in range(NPAIR + 1):
        if sp < NPAIR:
            if amplified:
                if sp > 0:
                    issue_at(sp, range(BL))  # no prefetch under For_i
            elif sp + 1 < NPAIR:
                issue_at(sp + 1, range(BL))  # prefetch next pair's annT
            an_tiles = []  # an_tiles[b][sub]
            for b in range(BL):
                subs = []
                for sub in range(2):
                    sc = 2 * sp + sub
                    t = annn_pool.tile([128, 4, A], bf16, tag=f"an{sub}{b}")
                    nc.sync.dma_start(
                        out=t,
                        in_=annN_d[b, sc * SC:(sc + 1) * SC, :].rearrange(
                            "(sb p) a -> p sb a", p=128
                        ),
                    )
                    subs.append(t)
                an_tiles.append(subs)

            th_tiles = [
                th_pool.tile(
                    [128, 4, 2, SC], bf16, tag=f"th{b}", name=f"th{b}"
                )
                for b in range(BL)
            ]
            score_ps = psum_sc.tile([128, 32], f32, tag="score")

        if pend is not None:
            p_w, p_an, p_sp = pend
            # one job per (ac, b) ctx column: its 8 st MMs are emitted
            # CONSECUTIVELY as a complete per-pair accumulation group (PSUM
            # allows only one open group per zero region at a time); the
            # per-pair partial is then folded into ctx_acc by the DVE
            ctx_ps = psum1.tile([128, BL, 4], f32, tag="ctx")
            ctx_jobs = [(ac, b) for ac in range(4) for b in range(BL)]
        else:
            ctx_jobs = []

        def emit_zp():
            # Z partials: column-sums of the previous pair's softmax
            # weights (ones-vector contraction over the s partitions),
            # then folded into zp_acc by the DVE
            zpp = psum_z.tile([32, 1], f32, tag="zpp")
            nc.tensor.matmul(
                zpp[:, :],
                lhsT=p_w[:, :],
                rhs=onesc_sb[:, :],
                start=True,
                stop=True,
            )
            nc.vector.tensor_tensor(
                out=zp_acc[:, :], in0=zp_acc[:, :], in1=zpp[:, :],
                op=mybir.AluOpType.add,
            )

        def emit_ctx(n):
            for _ in range(n):
                if not ctx_jobs:
                    return
                ac, b = ctx_jobs.pop(0)
                for st in range(8):
                    col = (st // 4) * 16 + (st % 4) * BL + b
                    nc.tensor.matmul(
                        ctx_ps[:, b, ac:ac + 1],
                        lhsT=p_an[b][st // 4][
                            :, st % 4, ac * 128:(ac + 1) * 128
                        ],
                        rhs=p_w[:, col:col + 1],
                        start=(st == 0),
                        stop=(st == 7),
                    )

        def fold_ctx():
            nc.vector.tensor_tensor(
                out=ctx_acc[:, :, :], in0=ctx_acc[:, :, :],
                in1=ctx_ps[:, :, :],
                op=mybir.AluOpType.add,
            )

        if sp < NPAIR:

            def emit_score(b):
                # scoreT[s, col] = sum_hc th[:, s].T @ w2[:, hc]; th is the
                # stationary operand, out free size is 1. Each column's 4
                # MMs are consecutive (complete group before the next).
                for sub in range(2):
                    for st in range(4):
                        col = sub * 16 + st * BL + b
                        for hc in range(4):
                            nc.tensor.matmul(
                                score_ps[:, col:col + 1],
                                lhsT=th_tiles[b][
                                    :, hc, sub, st * 128:(st + 1) * 128
                                ],
                                rhs=w2_sb[:, hc, :],
                                start=(hc == 0),
                                stop=(hc == 3),
                            )

            slot = 0
            for hc in range(4):
                for b in range(BL):
                    thp = psum_th.tile([128, 2, SC], f32, tag=f"thp{b % 2}")
                    for sub in range(2):
                        for kh in range(2):
                            nc.tensor.matmul(
                                thp[:, sub, :],
                                lhsT=w1a_sb[
                                    :, 2 * kh:2 * kh + 2,
                                    hc * 128:(hc + 1) * 128,
                                ],
                                rhs=at_tiles[(sp, sub, b)][
                                    :, 2 * kh:2 * kh + 2, :
                                ],
                                start=(kh == 0),
                                stop=(kh == 1),
                                perf_mode=DR,
                            )
                    nc.scalar.activation(
                        out=th_tiles[b][:, hc, :, :],
                        in_=thp[:, :, :],
                        func=Tanh,
                        bias=pre_sb[:, hc, b:b + 1],
                        scale=1.0 / W1A_SCALE,
                    )
                    if hc == 3 and b > 0:
                        emit_score(b - 1)
                    if pend is not None and slot == 0:
                        emit_zp()
                    if slot >= 2:
                        emit_ctx(2)
                    slot += 1
            emit_score(BL - 1)
            emit_ctx(len(ctx_jobs))
            if pend is not None:
                fold_ctx()

            # one exp over all 32 scoreT columns -> softmax weights with s
            # on partitions, directly consumable by the ctx matmuls
            w_sb = w_pool.tile([128, 32], bf16, tag="w")
            nc.scalar.activation(
                out=w_sb, in_=score_ps[:, :], func=Exp,
            )
            pend = (w_sb, an_tiles, sp)
        else:
            # drain: Z partials + remaining ctx MMs of the last pair
            emit_zp()
            emit_ctx(len(ctx_jobs))
            fold_ctx()
            pend = None


def _make_in_maps(prev_hidden_state, annotations, W1, b1, W2):
    prev_hidden_state = np.asarray(prev_hidden_state, dtype=np.float32)
    annotations = np.asarray(annotations, dtype=np.float32)
    W1 = np.asarray(W1, dtype=np.float32)
    b1 = np.asarray(b1, dtype=np.float32)
    W2 = np.asarray(W2, dtype=np.float32)

    annN = annotations.astype(BF16)
    annT = np.ascontiguousarray(annotations.transpose(0, 2, 1)).astype(F8)
    w1h = np.ascontiguousarray(W1[:H]).astype(BF16)
    w1a = np.ascontiguousarray(W1[H:] * W1A_SCALE).astype(F8)
    b1r = b1.reshape(1, H).astype(BF16)
    w2c = np.ascontiguousarray(W2.reshape(H, 1)).astype(BF16)
    sel = np.zeros((32, BL), dtype=np.float32)
    for k in range(32):
        sel[k, k % BL] = 1.0
    pvt = np.ascontiguousarray(prev_hidden_state.T).astype(BF16)  # [H, B]

    in_maps = []
    for c in range(NCORES):
        sl = slice(c * BL, (c + 1) * BL)
        in_maps.append(
            {
                "annT": np.ascontiguousarray(annT[sl]),
                "annN": np.ascontiguousarray(annN[sl]),
                "w1a": w1a,
                "w1h": w1h,
                "b1": b1r,
                "w2": w2c,
                "sel": sel,
                "pvt": np.ascontiguousarray(pvt[:, sl]),
            }
        )
    return in_maps


def kernel(prev_hidden_state, annotations, W1, b1, W2, b2, **_unused):
    global _BUILT, LAST_RESULT
    from concourse import bass_utils

    # b2 shifts every score equally; softmax is shift-invariant -> ignored.
    in_maps = _make_in_maps(prev_hidden_state, annotations, W1, b1, W2)

    if _BUILT is None:
        _BUILT = _build_bass()
    nc = _BUILT

    trace = bool(int(os.environ.get("KERNEL_TRACE", "0")))
    if not trace:
        # the NTFF trace path needs antenv.axon_hooks, absent in this
        # client -- make sure an ambient BASS_TRACE can't select it
        os.environ.setdefault("BASS_NEVER_TRACE", "1")
    res = bass_utils.run_bass_kernel_spmd(
        nc, in_maps, core_ids=list(range(NCORES)), trace=trace
    )
    LAST_RESULT = res
    out = np.concatenate([r["out"] for r in res.results], axis=0)  # [B, A]
    return out[:, None, :].astype(np.float32)


# revision 52
# speedup vs baseline: 2.2234x; 1.0253x over previous
"""Bass/Tile TRN2 kernel for the attention module:

    pre    = prev_hidden @ W1[:H] + b1                    [B, H]
    hidden = tanh(pre[:, None, :] + ann @ W1[H:])         [B, S, H]
    score  = hidden @ W2 (+ b2; softmax-invariant, drop)  [B, S]
    alpha  = softmax(score, axis=1)
    ctx    = alpha @ ann                                  [B, 1, A]

B=32, S=4096, A=H=512. Sharding: data-parallel over batch, 4 batches per
core on 8 cores. Single pass over S per batch with an unnormalized
online softmax (scores are bounded: |score| <= sum|W2|+|b2| ~ 11.4, so
exp never overflows in fp32 and no running-max is needed):

    w_s = exp(score_s);  Z = sum w_s;  ctx = (sum w_s * ann_s) / Z

Layouts: the s-dim matmul (ann @ W1a) contracts over the feature dim a,
so it needs ann with a on SBUF partitions (annT, fp8 e4m3); the context
matmul contracts over s, so it needs natural ann (annN, bf16). The
ann @ W1a matmul runs in fp8 DoubleRow mode (K=256 per MM, 0.5
cycles/row): W1a is host-prescaled x32 into e4m3's normal range and the
1/32 is folded into the tanh activation's scale. W1h (the tiny pre
matmul, on the startup critical path feeding every tanh's bias) is
likewise fp8 x32, rescaled in the PSUM->SBUF copy.

Schedule: s-chunks of 512 are processed in PAIRS so each tanh covers
1024 elements per partition (fewer Activation-engine calls). The score
is computed TRANSPOSED (s on partitions) by 1-column matmuls whose
stationary operand is the tanh tile and whose moving operand is the W2
vector, so exp directly yields softmax-weight columns in the layout the
ctx matmuls consume -- no weight transpose exists anywhere. The ctx
matmuls likewise make annN the stationary operand and stream one
w-column each (output free size 1), accumulating per-pair PSUM groups
that the otherwise-idle DVE folds into an SBUF accumulator (PSUM allows
only one open accumulation group per zero region, so kernel-lifetime
column groups are illegal). Z comes from a ones-vector contraction of
the weight columns plus a small selector matmul at the end. Within a
pair the in-order PE interleaves step2 (fp8 DR), the previous batch's
score groups, and the previous pair's ctx groups as filler, so it never
head-blocks on the Activation engine's tanh stream. Slots iterate
b-major (all four hc of one batch before the next batch) so at startup
the Activation engine streams batch 0's tanh groups while the other
batches' annT tiles are still arriving. annT DMAs are issued one pair
ahead of use; annN (first needed by the deferred ctx tail one pair
later) trails in the same queue.
"""

import os

import numpy as np
import ml_dtypes

B = 32
S = 4096
A = 512
H = 512
NCORES = 8
BL = B // NCORES  # 4 batches per core
SC = 512          # s-chunk per matmul moving operand
NSC = S // SC     # 8
NPAIR = NSC // 2  # chunk pairs per core

BF16 = ml_dtypes.bfloat16
F8 = ml_dtypes.float8_e4m3  # maps to mybir.dt.float8e4 (TRN fp8 e4m3)
W1A_SCALE = 32.0  # W1a entries ~U(+-1/32); prescale into e4m3's normal range

_BUILT = None       # (nc,) cache — Bass module is reusable across calls
LAST_RESULT = None  # last BassKernelResults, for test harness introspection


def _build_bass(loop_n=None):
    """Build the Bass module. loop_n wraps the main pair-loop in a For_i
    executed loop_n times — a timing amplifier (outputs then meaningless);
    loop_n=None builds the real single-pass kernel."""
    from contextlib import ExitStack, nullcontext

    import concourse.bass as bass
    import concourse.tile as tile
    from concourse import bacc, mybir
    from concourse.masks import make_identity

    bf16 = mybir.dt.bfloat16
    f8 = mybir.dt.float8e4
    f32 = mybir.dt.float32

    nc = bacc.Bacc()

    annT_d = nc.dram_tensor("annT", [BL, A, S], f8, kind="ExternalInput")
    annN_d = nc.dram_tensor("annN", [BL, S, A], bf16, kind="ExternalInput")
    w1a_d = nc.dram_tensor("w1a", [A, H], f8, kind="ExternalInput")
    w1h_d = nc.dram_tensor("w1h", [H, H], f8, kind="ExternalInput")
    b1_d = nc.dram_tensor("b1", [1, H], bf16, kind="ExternalInput")
    w2_d = nc.dram_tensor("w2", [H, 1], bf16, kind="ExternalInput")
    # sel[k, b] = 1 if k % BL == b: partition-group selector for the Z
    # reduction (sums the per-(sub,st) column partials of batch b)
    sel_d = nc.dram_tensor("sel", [32, BL], f32, kind="ExternalInput")
    pvt_d = nc.dram_tensor("pvt", [H, BL], bf16, kind="ExternalInput")
    out_d = nc.dram_tensor("out", [BL, A], f32, kind="ExternalOutput")

    with tile.TileContext(nc) as tc, ExitStack() as ctx:
        singles = ctx.enter_context(tc.tile_pool(name="singles", bufs=1))
        annt_pool = ctx.enter_context(tc.tile_pool(name="annt", bufs=2))
        annn_pool = ctx.enter_context(tc.tile_pool(name="annn", bufs=2))
        th_pool = ctx.enter_context(tc.tile_pool(name="thp", bufs=2))
        w_pool = ctx.enter_context(tc.tile_pool(name="wp", bufs=2))
        psum_th = ctx.enter_context(
            tc.tile_pool(name="psumth", bufs=1, space="PSUM")
        )
        psum_sc = ctx.enter_context(
            tc.tile_pool(name="psumsc", bufs=1, space="PSUM")
        )
        psum_z = ctx.enter_context(
            tc.tile_pool(name="psumz", bufs=1, space="PSUM")
        )
        psum1 = ctx.enter_context(
            tc.tile_pool(name="psum1", bufs=1, space="PSUM")
        )

        # ---- weights / constants into SBUF ----
        # DMA queue order matters: w1h/pvt/b1 first so the pre matmul (the
        # Activation engine's gating dependency for every tanh) lands as
        # early as possible; then w1a + the first pair's annT for the PE's
        # first step2 slots.
        w1h_sb = singles.tile([128, 4, H], f8)  # (hin%128, hin//128, h)
        nc.sync.dma_start(
            out=w1h_sb, in_=w1h_d[:, :].rearrange("(kc p) h -> p kc h", p=128)
        )
        pvt_sb = singles.tile([128, 4, BL], bf16)  # (hin%128, hin//128, b)
        nc.sync.dma_start(
            out=pvt_sb, in_=pvt_d[:, :].rearrange("(kc p) b -> p kc b", p=128)
        )
        b1_sb = singles.tile([1, H], bf16)
        nc.sync.dma_start(out=b1_sb, in_=b1_d[:, :])

        w1a_sb = singles.tile([128, 4, H], f8)  # (a%128, a//128, h)
        nc.sync.dma_start(
            out=w1a_sb, in_=w1a_d[:, :].rearrange("(ac p) h -> p ac h", p=128)
        )

        at_tiles = {}  # (sp, sub, b) -> tile, DMA-issued one pair ahead

        def issue_at(sp, bs):
            for b in bs:
                for sub in range(2):
                    sc = 2 * sp + sub
                    t = annt_pool.tile([128, 4, SC], f8, tag=f"at{sub}{b}")
                    nc.sync.dma_start(
                        out=t,
                        in_=annT_d[b, :, sc * SC:(sc + 1) * SC].rearrange(
                            "(ac p) s -> p ac s", p=128
                        ),
                    )
                    at_tiles[(sp, sub, b)] = t

        issue_at(0, (0, 1))
        issue_at(0, (2, 3))

        w2_sb = singles.tile([128, 4, 1], bf16)  # (h%128, h//128, 1)
        nc.sync.dma_start(
            out=w2_sb, in_=w2_d[:, :].rearrange("(hc p) r -> p hc r", p=128)
        )
        sel_sb = singles.tile([32, BL], f32)
        nc.sync.dma_start(out=sel_sb, in_=sel_d[:, :])

        ones_sb = singles.tile([1, BL], bf16)
        nc.vector.memset(ones_sb, 1.0)
        onesc_sb = singles.tile([128, 1], bf16)
        nc.vector.memset(onesc_sb, 1.0)
        onesr_sb = singles.tile([1, 128], f32)
        nc.vector.memset(onesr_sb, 1.0)
        ones32_sb = singles.tile([32, 1], f32)
        nc.vector.memset(ones32_sb, 1.0)
        identf = singles.tile([128, 128], f32)
        make_identity(nc, identf)

        pre_sb = singles.tile([128, 4, BL], f32)

        def emit_pre():
            # pre2T[h, b] = (prev @ W1h).T + b1 broadcast.
            pre_ps = psum_sc.tile([128, 32], f32, tag="score")
            for hc in range(4):
                for kc in range(4):
                    nc.tensor.matmul(
                        pre_ps[:, hc * BL:(hc + 1) * BL],
                        lhsT=w1h_sb[:, kc, hc * 128:(hc + 1) * 128],
                        rhs=pvt_sb[:, kc, :],
                        start=(kc == 0),
                        stop=False,
                    )
                # b1 contribution: rank-1 with ones row (K=1)
                nc.tensor.matmul(
                    pre_ps[:, hc * BL:(hc + 1) * BL],
                    lhsT=b1_sb[:, hc * 128:(hc + 1) * 128],
                    rhs=ones_sb[:, :],
                    start=False,
                    stop=True,
                )
            nc.scalar.activation(
                out=pre_sb,
                in_=pre_ps[:, 0:4 * BL].rearrange(
                    "p (hc b) -> p hc b", b=BL
                ),
                func=mybir.ActivationFunctionType.Identity,
                scale=1.0 / W1A_SCALE,
            )

        ctx_acc = singles.tile([128, BL, 4], f32)  # (a%128, b, ac)
        nc.vector.memset(ctx_acc, 0.0)
        zp_acc = singles.tile([32, 1], f32)  # per-(sub,st,b) Z partials
        nc.vector.memset(zp_acc, 0.0)

        outer = (
            tc.For_i(0, loop_n, 1) if loop_n is not None else nullcontext()
        )
        with outer:
            _main_body(
                nc, tc, mybir,
                annN_d, w1a_sb, w2_sb, pre_sb,
                at_tiles, issue_at, emit_pre,
                annn_pool, th_pool, w_pool, psum_th, psum_sc, psum_z,
                psum1, onesc_sb, ctx_acc, zp_acc,
                amplified=loop_n is not None,
            )

        # ---- normalize and store ----
        # Z per batch: group-sum the [32,1] column partials via the selector
        # matmul, reciprocal, transpose to a row, broadcast over partitions,
        # then scale ctx and store.
        zpsel_sb = singles.tile([32, BL], f32)
        nc.vector.tensor_scalar_mul(zpsel_sb, sel_sb[:, :], zp_acc[:, 0:1])
        zbt_ps = psum_sc.tile([1, BL], f32, tag="score")
        nc.tensor.matmul(
            zbt_ps[:, :], lhsT=ones32_sb[:, :], rhs=zpsel_sb[:, :],
            start=True, stop=True,
        )
        zrt_sb = singles.tile([1, BL], f32)
        nc.vector.reciprocal(out=zrt_sb, in_=zbt_ps)
        bc_ps = psum_sc.tile([128, BL], f32, tag="score")
        nc.tensor.matmul(
            bc_ps[:, :], lhsT=onesr_sb[:, :], rhs=zrt_sb[:, :],
            start=True, stop=True,
        )
        out_sb = singles.tile([128, BL, 4], f32)
        for b in range(BL):
            nc.vector.tensor_scalar_mul(
                out_sb[:, b, :], ctx_acc[:, b, :], bc_ps[:, b:b + 1]
            )
        outt_ps = psum_sc.tile([4 * BL, 128], f32, tag="score")
        nc.tensor.transpose(
            outt_ps[:, :],
            out_sb[:, :, :].rearrange("p b ac -> p (b ac)"),
            identf[:, :],
        )
        outt_sb = singles.tile([4 * BL, 128], f32)
        nc.vector.tensor_copy(out=outt_sb, in_=outt_ps[:, :])
        nc.sync.dma_start(
            out=out_d[:, :].rearrange("b (ac p) -> (b ac) p", p=128),
            in_=outt_sb,
        )

    nc.finalize()
    return nc


def _main_body(
    nc, tc, mybir,
    annN_d, w1a_sb, w2_sb, pre_sb,
    at_tiles, issue_at, emit_pre,
    annn_pool, th_pool, w_pool, psum_th, psum_sc, psum_z,
    psum1, onesc_sb, ctx_acc, zp_acc,
    amplified=False,
):
    bf16 = mybir.dt.bfloat16
    f8 = mybir.dt.float8e4
    f32 = mybir.dt.float32
    Tanh = mybir.ActivationFunctionType.Tanh
    Exp = mybir.ActivationFunctionType.Exp
    DR = mybir.MatmulPerfMode.DoubleRow

    # Score is computed TRANSPOSED (s on partitions): scoreT[s, col] with
    # col = sub*16 + st*4 + b, via tiny 1-column matmuls whose STATIONARY
    # operand is the th tile (lhsT) and whose moving operand is the W2
    # vector. exp then produces the softmax weights already in the layout
    # the ctx matmuls need as moving columns, so no w transpose exists at
    # all. The ctx matmuls likewise make annN the stationary operand and
    # stream one w column (out free size 1). The ctx MMs of pair sp-1 are
    # deferred into iteration sp and spread through its slots as filler.
    emit_pre()
    pend = None
    for sp in range(NPAIR + 1):
        if sp < NPAIR:
            if amplified:
                if sp > 0:
                    issue_at(sp, range(BL))  # no prefetch under For_i
            elif sp + 1 < NPAIR:
                issue_at(sp + 1, range(BL))  # prefetch next pair's annT
            an_tiles = []  # an_tiles[b][sub]
            for b in range(BL):
                subs = []
                for sub in range(2):
                    sc = 2 * sp + sub
                    t = annn_pool.tile([128, 4, A], bf16, tag=f"an{sub}{b}")
                    nc.sync.dma_start(
                        out=t,
                        in_=annN_d[b, sc * SC:(sc + 1) * SC, :].rearrange(
                            "(sb p) a -> p sb a", p=128
                        ),
                    )
                    subs.append(t)
                an_tiles.append(subs)

            th_tiles = [
                th_pool.tile(
                    [128, 4, 2, SC], bf16, tag=f"th{b}", name=f"th{b}"
                )
                for b in range(BL)
            ]
            score_ps = psum_sc.tile([128, 32], f32, tag="score")

        if pend is not None:
            p_w, p_an, p_sp = pend
            # one job per (ac, b) ctx column: its 8 st MMs are emitted
            # CONSECUTIVELY as a complete per-pair accumulation group (PSUM
            # allows only one open group per zero region at a time); the
            # per-pair partial is then folded into ctx_acc by the DVE
            ctx_ps = psum1.tile([128, BL, 4], f32, tag="ctx")
            ctx_jobs = [(ac, b) for ac in range(4) for b in range(BL)]
        else:
            ctx_jobs = []

        def emit_zp():
            # Z partials: column-sums of the previous pair's softmax
            # weights (ones-vector contraction over the s partitions),
            # then folded into zp_acc by the DVE
            zpp = psum_z.tile([32, 1], f32, tag="zpp")
            nc.tensor.matmul(
                zpp[:, :],
                lhsT=p_w[:, :],
                rhs=onesc_sb[:, :],
                start=True,
                stop=True,
            )
            nc.vector.tensor_tensor(
                out=zp_acc[:, :], in0=zp_acc[:, :], in1=zpp[:, :],
                op=mybir.AluOpType.add,
            )

        def emit_ctx(n):
            for _ in range(n):
                if not ctx_jobs:
                    return
                ac, b = ctx_jobs.pop(0)
                for st in range(8):
                    col = (st // 4) * 16 + (st % 4) * BL + b
                    nc.tensor.matmul(
                        ctx_ps[:, b, ac:ac + 1],
                        lhsT=p_an[b][st // 4][
                            :, st % 4, ac * 128:(ac + 1) * 128
                        ],
                        rhs=p_w[:, col:col + 1],
                        start=(st == 0),
                        stop=(st == 7),
                    )

        def fold_ctx():
            nc.vector.tensor_tensor(
                out=ctx_acc[:, :, :], in0=ctx_acc[:, :, :],
                in1=ctx_ps[:, :, :],
                op=mybir.AluOpType.add,
            )

        if sp < NPAIR:

            def emit_score(b):
                # scoreT[s, col] = sum_hc th[:, s].T @ w2[:, hc]; th is the
                # stationary operand, out free size is 1. Each column's 4
                # MMs are consecutive (complete group before the next).
                for sub in range(2):
                    for st in range(4):
                        col = sub * 16 + st * BL + b
                        for hc in range(4):
                            nc.tensor.matmul(
                                score_ps[:, col:col + 1],
                                lhsT=th_tiles[b][
                                    :, hc, sub, st * 128:(st + 1) * 128
                                ],
                                rhs=w2_sb[:, hc, :],
                                start=(hc == 0),
                                stop=(hc == 3),
                            )

            slot = 0
            for b in range(BL):
                for hc in range(4):
                    thp = psum_th.tile([128, 2, SC], f32, tag=f"thp{hc % 2}")
                    for sub in range(2):
                        for kh in range(2):
                            nc.tensor.matmul(
                                thp[:, sub, :],
                                lhsT=w1a_sb[
                                    :, 2 * kh:2 * kh + 2,
                                    hc * 128:(hc + 1) * 128,
                                ],
                                rhs=at_tiles[(sp, sub, b)][
                                    :, 2 * kh:2 * kh + 2, :
                                ],
                                start=(kh == 0),
                                stop=(kh == 1),
                                perf_mode=DR,
                            )
                    nc.scalar.activation(
                        out=th_tiles[b][:, hc, :, :],
                        in_=thp[:, :, :],
                        func=Tanh,
                        bias=pre_sb[:, hc, b:b + 1],
                        scale=1.0 / W1A_SCALE,
                    )
                    if hc == 0 and b > 0:
                        emit_score(b - 1)
                    if pend is not None and slot == 0:
                        emit_zp()
                    if slot >= 2:
                        emit_ctx(2)
                    slot += 1
            emit_score(BL - 1)
            emit_ctx(len(ctx_jobs))
            if pend is not None:
                fold_ctx()

            # one exp over all 32 scoreT columns -> softmax weights with s
            # on partitions, directly consumable by the ctx matmuls
            w_sb = w_pool.tile([128, 32], bf16, tag="w")
            nc.scalar.activation(
                out=w_sb, in_=score_ps[:, :], func=Exp,
            )
            pend = (w_sb, an_tiles, sp)
        else:
            # drain: Z partials + remaining ctx MMs of the last pair
            emit_zp()
            emit_ctx(len(ctx_jobs))
            fold_ctx()
            pend = None


def _make_in_maps(prev_hidden_state, annotations, W1, b1, W2):
    prev_hidden_state = np.asarray(prev_hidden_state, dtype=np.float32)
    annotations = np.asarray(annotations, dtype=np.float32)
    W1 = np.asarray(W1, dtype=np.float32)
    b1 = np.asarray(b1, dtype=np.float32)
    W2 = np.asarray(W2, dtype=np.float32)

    annN = annotations.astype(BF16)
    annT = np.ascontiguousarray(annotations.transpose(0, 2, 1)).astype(F8)
    w1h = np.ascontiguousarray(W1[:H] * W1A_SCALE).astype(F8)
    w1a = np.ascontiguousarray(W1[H:] * W1A_SCALE).astype(F8)
    b1r = (b1 * W1A_SCALE).reshape(1, H).astype(BF16)
    w2c = np.ascontiguousarray(W2.reshape(H, 1)).astype(BF16)
    sel = np.zeros((32, BL), dtype=np.float32)
    for k in range(32):
        sel[k, k % BL] = 1.0
    pvt = np.ascontiguousarray(prev_hidden_state.T).astype(BF16)  # [H, B]

    in_maps = []
    for c in range(NCORES):
        sl = slice(c * BL, (c + 1) * BL)
        in_maps.append(
            {
                "annT": np.ascontiguousarray(annT[sl]),
                "annN": np.ascontiguousarray(annN[sl]),
                "w1a": w1a,
                "w1h": w1h,
                "b1": b1r,
                "w2": w2c,
                "sel": sel,
                "pvt": np.ascontiguousarray(pvt[:, sl]),
            }
        )
    return in_maps


def kernel(prev_hidden_state, annotations, W1, b1, W2, b2, **_unused):
    global _BUILT, LAST_RESULT
    from concourse import bass_utils

    # b2 shifts every score equally; softmax is shift-invariant -> ignored.
    in_maps = _make_in_maps(prev_hidden_state, annotations, W1, b1, W2)

    if _BUILT is None:
        _BUILT = _build_bass()
    nc = _BUILT

    trace = bool(int(os.environ.get("KERNEL_TRACE", "0")))
    if not trace:
        # the NTFF trace path needs antenv.axon_hooks, absent in this
        # client -- make sure an ambient BASS_TRACE can't select it
        os.environ.setdefault("BASS_NEVER_TRACE", "1")
    res = bass_utils.run_bass_kernel_spmd(
        nc, in_maps, core_ids=list(range(NCORES)), trace=trace
    )
    LAST_RESULT = res
    out = np.concatenate([r["out"] for r in res.results], axis=0)  # [B, A]
    return out[:, None, :].astype(np.float32)
